# revision 1
# baseline (speedup 1.0000x reference)
"""Trainium2 Bass kernel for nn_Decoder — dual-column GRU decoder.

Design ("DUAL-9")
-----------------
Data-parallel over batch: 8 cores x 8 batch rows -> 32768 columns per core.
Columns are packed TWO-PER-PARTITION-PAIR: a "dchunk" is 1024 columns stored
as a [128, 512] tile — columns 0:512 on partitions 0:64 ("lo"), columns
512:1024 on partitions 64:128 ("hi").  All gate matmuls use block-diagonal
lhsT [128,128] = diag(W.T, W.T), so one N=512 stream computes gates for 1024
columns, and every elementwise/activation op runs at full 128-partition
width.

Per step t and dchunk (PSUM tiles dual-packed [128, 512] f32):
  az   = blockdiag(Az.T)  @ h   + X_z @ xrows      (Az = Wz + Gz0*wo' fold)
  ar   = blockdiag(Ar.T)  @ h   + X_r @ xrows
  z, r = sigmoid(az), sigmoid(ar)                  (ACT, psum->sbuf bf16)
  hn   = blockdiag(Wn.T)  @ h                      (PSUM P)
  v    = (hn + b_hhn) * r                          (DVE scalar_tensor_tensor)
  np   = blockdiag(Fn.T)@h + X_n @ xrows + I @ v   (PSUM P reused; Fn = i_n
                                                    feedback outer(Gn0, wo))
  n    = tanh(np)                                  (ACT)
  h'   = n + z*(h - n)                             (3 DVE ops, pair-wide)
  pred = [wo;0 | 0;wo].T @ h'  (+bo at evacuation)

x-rows per (dchunk, t): [12, 512] = per half [xt(3); xn(t=0); 1; bo-step].
The x_prev feedback is folded into the h-weights (t>=1) / carried by the
xn row (t=0), so no per-step scalar evacuation is needed.

Preds accumulate in one shared PSUM bank [128, 512] per 4-dchunk group
(row = 32*d' + 2*(t%8) + half), evacuated to SBUF every 8 steps by one DVE
tensor_scalar (+bo), and DMA'd out once per group as [128, 3072] bf16.
"""

import os

import numpy as np

import concourse.bass as bass
import concourse.mybir as mybir
import concourse.tile as tile
from concourse import bacc
from concourse.bass_utils import run_bass_kernel_spmd

F32 = mybir.dt.float32
BF16 = mybir.dt.bfloat16
FP8 = mybir.dt.float8e4
AF = mybir.ActivationFunctionType
ALU = mybir.AluOpType

B, T_HIST, T_FC, C, F_IN, HID = 64, 24, 48, 4096, 8, 64
N_CORES = 8
B_LOC = B // N_CORES
NCOLS = B_LOC * C            # 32768 columns per core
DCH = 1024                   # columns per dchunk (dual-packed)
NDCH = NCOLS // DCH          # 32 dchunks
GRP = 4                      # dchunks per group (pred psum packing)
NGRP = NDCH // GRP           # 8 groups
NEP = T_FC // 8              # 6 pred epochs of 8 steps

_BUILT = {}
LAST_RESULTS = None

W128 = ["AZ1", "AR1", "AZ0", "AR0", "WN", "FN", "EYE"]


def _build():
    if "nc" in _BUILT:
        return _BUILT["nc"]

    nc = bacc.Bacc("TRN2", target_bir_lowering=False, debug=False,
                   num_devices=N_CORES)

    d_ht = nc.dram_tensor("HT", [NDCH, 128, 512], BF16,
                          kind="ExternalInput").ap()
    d_xt = nc.dram_tensor("XT", [NDCH, T_FC, 12, 512], BF16,
                          kind="ExternalInput").ap()
    d_w = {}
    for name in W128:
        d_w[name] = nc.dram_tensor(name, [128, 128], BF16,
                                   kind="ExternalInput").ap()
    for name in ("XZ", "XR", "XN"):
        d_w[name] = nc.dram_tensor(name, [12, 128], BF16,
                                   kind="ExternalInput").ap()
    d_w["WO32"] = nc.dram_tensor("WO32", [128, 256], BF16,
                                 kind="ExternalInput").ap()
    d_w["BNH"] = nc.dram_tensor("BNH", [128, 1], F32,
                                kind="ExternalInput").ap()
    d_out = nc.dram_tensor("OUT", [NGRP, 128, 512 * NEP], BF16,
                           kind="ExternalOutput").ap()

    with tile.TileContext(nc) as tc:
        with (
            tc.tile_pool(name="wpool", bufs=1) as wpool,
            tc.tile_pool(name="xpool", bufs=1) as xpool,
            tc.tile_pool(name="hpool", bufs=1) as hpool,
            tc.tile_pool(name="tpool", bufs=1) as tpool,
            tc.tile_pool(name="opool", bufs=1) as opool,
            tc.tile_pool(name="pspool", bufs=1, space="PSUM") as pspool,
        ):
            w = {}
            for name, ap in d_w.items():
                wt = wpool.tile(list(ap.shape), ap.dtype, name=f"w_{name}")
                nc.gpsimd.dma_start(wt[:], ap[:])
                w[name] = wt

            for g in range(NGRP):
                dbase = g * GRP
                # --- initial h + first x tiles for the 2 pairs ---
                hcur = {}
                for pr in range(2):
                    ht = hpool.tile([128, DCH], BF16, tag=f"h{pr}", bufs=3,
                                    name="ht")
                    for dd in range(2):
                        d = dbase + 2 * pr + dd
                        nc.gpsimd.dma_start(ht[:, 512 * dd:512 * (dd + 1)],
                                            d_ht[d])
                    hcur[pr] = ht

                xts = {}
                XPF = 3      # x prefetch lead (steps)
                for t0 in range(XPF):
                    for dp in range(GRP):
                        xt = xpool.tile([12, 512], BF16, tag=f"x{dp}",
                                        bufs=XPF + 2, name="xt")
                        nc.gpsimd.dma_start(xt[:], d_xt[dbase + dp, t0])
                        xts[(dp, t0)] = xt

                outstage = opool.tile([128, 512 * NEP], BF16, tag="ost",
                                      bufs=2, name="outstage")

                pending_preds = []   # (d_local, h_tile, dd)

                def flush_preds(predp):
                    t8 = flush_preds.t8
                    for (dl, htile, dd) in pending_preds:
                        nc.tensor.matmul(
                            predp[32 * dl:32 * dl + 32, :],
                            w["WO32"][:, 32 * t8:32 * t8 + 32],
                            htile[:, 512 * dd:512 * (dd + 1)],
                            start=(t8 == 0), stop=(t8 == 7),
                            tile_position=(0, 32 * dl))
                    pending_preds.clear()

                predp = None
                for t in range(T_FC):
                    if t % 8 == 0:
                        predp = pspool.tile([128, 512], F32, tag="pred",
                                            bufs=2, name="predp")
                    # prefetch x tiles for step t+XPF
                    tp = t + XPF
                    if tp < T_FC:
                        for dp in range(GRP):
                            xt = xpool.tile([12, 512], BF16, tag=f"x{dp}",
                                            bufs=XPF + 2, name="xt")
                            nc.gpsimd.dma_start(xt[:], d_xt[dbase + dp, tp])
                            xts[(dp, tp)] = xt

                    for pr in range(2):
                        d0, d1 = 2 * pr, 2 * pr + 1
                        ht = hcur[pr]
                        x0 = xts.pop((d0, t))
                        x1 = xts.pop((d1, t))

                        azw = w["AZ1"] if t > 0 else w["AZ0"]
                        arw = w["AR1"] if t > 0 else w["AR0"]

                        az, ar, P = {}, {}, {}
                        for dd in (0, 1):
                            az[dd] = pspool.tile([128, 512], F32, tag="az",
                                                 bufs=2, name="az")
                        # az h-part then x-part (batched by lhsT)
                        for dd in (0, 1):
                            nc.tensor.matmul(
                                az[dd][:], azw[:],
                                ht[:, 512 * dd:512 * (dd + 1)],
                                start=True, stop=False)
                        for dd, xx in ((0, x0), (1, x1)):
                            nc.tensor.matmul(az[dd][:], w["XZ"][:], xx[:],
                                             start=False, stop=True)
                        # sigmoid(az) -> zs halves
                        zs = tpool.tile([128, DCH], BF16, tag=f"zs{pr}",
                                        bufs=2, name="zs")
                        for dd in (0, 1):
                            nc.scalar.activation(
                                zs[:, 512 * dd:512 * (dd + 1)], az[dd][:],
                                AF.Sigmoid)

                        for dd in (0, 1):
                            ar[dd] = pspool.tile([128, 512], F32, tag="ar",
                                                 bufs=2, name="ar")
                        for dd in (0, 1):
                            nc.tensor.matmul(
                                ar[dd][:], arw[:],
                                ht[:, 512 * dd:512 * (dd + 1)],
                                start=True, stop=False)
                        for dd, xx in ((0, x0), (1, x1)):
                            nc.tensor.matmul(ar[dd][:], w["XR"][:], xx[:],
                                             start=False, stop=True)
                        rs = tpool.tile([128, DCH], BF16, tag=f"rs{pr}",
                                        bufs=2, name="rs")
                        for dd in (0, 1):
                            nc.scalar.activation(
                                rs[:, 512 * dd:512 * (dd + 1)], ar[dd][:],
                                AF.Sigmoid)

                        # hn into P
                        for dd in (0, 1):
                            P[dd] = pspool.tile([128, 512], F32, tag="P",
                                                bufs=2, name="P")
                        for dd in (0, 1):
                            nc.tensor.matmul(
                                P[dd][:], w["WN"][:],
                                ht[:, 512 * dd:512 * (dd + 1)],
                                start=True, stop=True)
                        # v = (hn + bnh) * r   (DVE STT, psum 1x)
                        vp = tpool.tile([128, DCH], BF16, tag=f"vp{pr}",
                                        bufs=2, name="vp")
                        for dd in (0, 1):
                            nc.vector.scalar_tensor_tensor(
                                vp[:, 512 * dd:512 * (dd + 1)], P[dd][:],
                                w["BNH"][:], rs[:, 512 * dd:512 * (dd + 1)],
                                op0=ALU.add, op1=ALU.mult)
                        # np = Fn@h + Xn@x + I@v  (P reused)
                        if t > 0:
                            for dd in (0, 1):
                                nc.tensor.matmul(
                                    P[dd][:], w["FN"][:],
                                    ht[:, 512 * dd:512 * (dd + 1)],
                                    start=True, stop=False)
                            for dd, xx in ((0, x0), (1, x1)):
                                nc.tensor.matmul(P[dd][:], w["XN"][:], xx[:],
                                                 start=False, stop=False)
                        else:
                            for dd, xx in ((0, x0), (1, x1)):
                                nc.tensor.matmul(P[dd][:], w["XN"][:], xx[:],
                                                 start=True, stop=False)
                        for dd in (0, 1):
                            nc.tensor.matmul(
                                P[dd][:], w["EYE"][:],
                                vp[:, 512 * dd:512 * (dd + 1)],
                                start=False, stop=True)
                        # n = tanh(np)
                        nt = tpool.tile([128, DCH], BF16, tag=f"nt{pr}",
                                        bufs=2, name="nt")
                        for dd in (0, 1):
                            nc.scalar.activation(
                                nt[:, 512 * dd:512 * (dd + 1)], P[dd][:],
                                AF.Tanh)
                        # h' = n + z*(h-n)   pair-wide [128, 1024]
                        hm = tpool.tile([128, DCH], BF16, tag=f"hm{pr}",
                                        bufs=2, name="hm")
                        nc.vector.tensor_tensor(hm[:], ht[:], nt[:],
                                                op=ALU.subtract)
                        ztt = tpool.tile([128, DCH], BF16, tag=f"zt{pr}",
                                         bufs=2, name="ztt")
                        nc.vector.tensor_tensor(ztt[:], zs[:], hm[:],
                                                op=ALU.mult)
                        hnew = hpool.tile([128, DCH], BF16, tag=f"h{pr}",
                                          bufs=3, name="hnew")
                        nc.vector.tensor_tensor(hnew[:], nt[:], ztt[:],
                                                op=ALU.add)
                        hcur[pr] = hnew

                        # preds for this pair (lagged emission)
                        flush_preds.t8 = t % 8
                        for dd, dl in ((0, d0), (1, d1)):
                            pending_preds.append((dl, hnew, dd))
                        if pr == 1:
                            flush_preds(predp)

                    if t % 8 == 7:
                        ep = t // 8
                        nc.vector.tensor_scalar_add(
                            outstage[:, 512 * ep:512 * (ep + 1)],
                            predp[:], _BUILT["bo"])
                # DMA group output
                nc.gpsimd.dma_start(d_out[g], outstage[:])

    nc.compile()
    _BUILT["nc"] = nc
    return nc


def _prep_weights(W_in, b_in, W_ih, W_hh, b_ih, b_hh, W_out, b_out):
    import ml_dtypes
    f8 = np.float64
    G = W_ih.astype(f8) @ W_in.astype(f8)     # [192, 4]
    c = W_ih.astype(f8) @ b_in.astype(f8) + b_ih
    Wr, Wz, Wn = (W_hh[0:64].astype(f8), W_hh[64:128].astype(f8),
                  W_hh[128:192].astype(f8))
    brh, bzh, bnh = (b_hh[0:64].astype(f8), b_hh[64:128].astype(f8),
                     b_hh[128:192].astype(f8))
    cr, cz, cn = c[0:64], c[64:128], c[128:192]
    Gr0, Gz0, Gn0 = G[0:64, 0], G[64:128, 0], G[128:192, 0]
    Grx, Gzx, Gnx = G[0:64, 1:4], G[64:128, 1:4], G[128:192, 1:4]
    wo = W_out.astype(f8)[0]
    bo = float(b_out[0])

    Az = Wz + np.outer(Gz0, wo)
    Ar = Wr + np.outer(Gr0, wo)
    Fn = np.outer(Gn0, wo)
    dz0, dr0 = cz + bzh, cr + brh

    def bd(m):   # blockdiag of m.T ([64,64] -> [128,128] lhsT)
        out = np.zeros((128, 128), f8)
        out[0:64, 0:64] = m.T
        out[64:128, 64:128] = m.T
        return out

    def _wo32(wo):   # [128, 256]: 8 variants of [128,32] pred lhsT
        out = np.zeros((128, 256), f8)
        for t8 in range(8):
            out[0:64, 32 * t8 + 2 * t8] = wo
            out[64:128, 32 * t8 + 2 * t8 + 1] = wo
        return out

    def xw(Gx, G0, d):   # [12, 128] x-side lhsT
        blk = np.stack([Gx[:, 0], Gx[:, 1], Gx[:, 2], G0, d, G0 * bo],
                       axis=0)  # [6, 64]
        out = np.zeros((12, 128), f8)
        out[0:6, 0:64] = blk
        out[6:12, 64:128] = blk
        return out

    w = {
        "AZ1": bd(Az), "AR1": bd(Ar), "AZ0": bd(Wz), "AR0": bd(Wr),
        "WN": bd(Wn), "FN": bd(Fn), "EYE": np.eye(128),
        "XZ": xw(Gzx, Gz0, dz0), "XR": xw(Grx, Gr0, dr0),
        "XN": xw(Gnx, Gn0, cn),
        "WO32": _wo32(wo),
        "BNH": np.concatenate([bnh, bnh])[:, None],
    }
    out = {}
    for k, v in w.items():
        dt = np.float32 if k == "BNH" else ml_dtypes.bfloat16
        out[k] = np.ascontiguousarray(v.astype(dt))
    _BUILT["bo"] = bo
    return out


def kernel(X, H, xn, W_in, b_in, W_ih, W_hh, b_ih, b_hh, W_out, b_out):
    global LAST_RESULTS
    import ml_dtypes
    X = np.asarray(X, np.float32)
    H = np.asarray(H, np.float32)
    xn = np.asarray(xn, np.float32)
    wmap = _prep_weights(np.asarray(W_in), np.asarray(b_in),
                         np.asarray(W_ih), np.asarray(W_hh),
                         np.asarray(b_ih), np.asarray(b_hh),
                         np.asarray(W_out), np.asarray(b_out))

    Xs = X[:, T_HIST:T_HIST + T_FC, :, F_IN - 3:F_IN]   # [B, 48, C, 3]

    in_maps = []
    for ci in range(N_CORES):
        bs = slice(ci * B_LOC, (ci + 1) * B_LOC)
        # columns: b_loc*C + city ; dchunk d covers cols 1024d..1024d+1024
        Xc = np.transpose(Xs[bs], (1, 0, 2, 3)).reshape(T_FC, NCOLS, 3)
        xnc = xn[bs, :, 0].reshape(NCOLS)
        Hc = H[bs].reshape(NCOLS, HID)

        HT = np.empty((NDCH, 128, 512), np.float32)
        XT = np.zeros((NDCH, T_FC, 12, 512), np.float32)
        for d in range(NDCH):
            for half in range(2):
                cs = slice(d * DCH + 512 * half, d * DCH + 512 * (half + 1))
                HT[d, 64 * half:64 * half + 64] = Hc[cs].T
                o = 6 * half
                XT[d, :, o:o + 3, :] = np.transpose(Xc[:, cs, :], (0, 2, 1))
                XT[d, 0, o + 3, :] = xnc[cs]       # xn row (t=0 only)
                XT[d, :, o + 4, :] = 1.0           # bias row
                XT[d, 1:, o + 5, :] = 1.0          # bo-step row (t>=1)
        m = {"HT": HT.astype(ml_dtypes.bfloat16),
             "XT": XT.astype(ml_dtypes.bfloat16)}
        m.update(wmap)
        in_maps.append(m)

    nc = _build()

    trace = os.environ.get("BASS_KERNEL_TRACE") == "1"
    if trace:
        _register_ntff_hook()
    res = run_bass_kernel_spmd(nc, in_maps, list(range(N_CORES)),
                               trace=trace)
    LAST_RESULTS = res

    out = np.empty((B, T_FC, C, 1), np.float32)
    t8 = np.arange(T_FC)
    for ci in range(N_CORES):
        O = res.results[ci]["OUT"].astype(np.float32)  # [NGRP,128,512*NEP]
        O = O.reshape(NGRP, 128, NEP, 512)
        core = np.empty((T_FC, NCOLS), np.float32)
        for gidx in range(NGRP):
            for dl in range(GRP):
                d = gidx * GRP + dl
                for half in range(2):
                    cs = slice(d * DCH + 512 * half,
                               d * DCH + 512 * (half + 1))
                    core[:, cs] = O[gidx, 32 * dl + 2 * (t8 % 8) + half,
                                    t8 // 8, :]
        bs = slice(ci * B_LOC, (ci + 1) * B_LOC)
        out[bs] = core.reshape(T_FC, B_LOC, C, 1).transpose(1, 0, 2, 3)
    return out


def _register_ntff_hook():
    import sys
    import types
    if "antenv.axon_hooks" in sys.modules:
        return
    mod = types.ModuleType("antenv.axon_hooks")
    state = {"hook": None}
    mod.set_axon_ntff_profile_hook = lambda h: state.update(hook=h)
    mod.get_axon_ntff_profile_hook = lambda: state["hook"]
    sys.modules["antenv.axon_hooks"] = mod
    try:
        import antenv
        antenv.axon_hooks = mod
    except ImportError:
        pass
    try:
        from trn_agent_boot.trn_boot import _ntff_profile_via_ctypes
        hook = _ntff_profile_via_ctypes("/opt/axon/libaxon_pjrt.so")
        if hook is not None:
            mod.set_axon_ntff_profile_hook(hook)
    except Exception as e:  # pragma: no cover
        print(f"NTFF hook registration failed: {e}")
    import concourse.bass_utils as bu
    bu.upload_artifacts = lambda tmpdir: f"file://{tmpdir}"

